# revision 11
# baseline (speedup 1.0000x reference)
import sys
sys.path.insert(0, '/opt/trn_rl_repo')
import numpy as np
from concourse import bass, tile, bacc, mybir
from concourse.bass_utils import run_bass_kernel_spmd

F32 = mybir.dt.float32
BF16 = mybir.dt.bfloat16
AF = mybir.ActivationFunctionType
ALU = mybir.AluOpType

B, N, DIN, DMID, DOUT = 512, 8192, 128, 256, 16
NCORES = 8
BC = B // NCORES            # 64 queries per core
NT = N // 128               # 64 n-tiles
FD = DOUT * BC              # 1024 free elements, col = d*64 + b


def build_kernel(reps=1):
    nc = bacc.Bacc(None, target_bir_lowering=False)

    x_d = nc.dram_tensor("x_slab", [BC, DIN], F32, kind="ExternalInput")
    X_d = nc.dram_tensor("calc_X", [N, DIN], F32, kind="ExternalInput")
    Y_d = nc.dram_tensor("calc_Y", [N, DOUT], F32, kind="ExternalInput")
    W1_d = nc.dram_tensor("W1", [DMID, DIN], F32, kind="ExternalInput")
    W2_d = nc.dram_tensor("W2", [DOUT, DMID], F32, kind="ExternalInput")
    hinv_d = nc.dram_tensor("hinv128", [128, 1], F32, kind="ExternalInput")
    mask_d = nc.dram_tensor("mask_db", [DOUT, FD], F32, kind="ExternalInput")
    out_d = nc.dram_tensor("y_out", [BC, DOUT], F32, kind="ExternalOutput")

    with tile.TileContext(nc) as tc:
      for _rep in range(reps):
        with (
            tc.tile_pool(name="dram", bufs=1, space="DRAM") as dram,
            tc.tile_pool(name="const", bufs=1) as cpool,
            tc.tile_pool(name="big", bufs=1) as big,
        ):
            # ---------- bf16 casts via HBM round trip ----------
            Xbf = dram.tile([N, DIN], BF16)
            xbf = dram.tile([BC, DIN], BF16)
            W1bf = dram.tile([DMID, DIN], BF16)
            W2bf = dram.tile([DOUT, DMID], BF16)
            nc.gpsimd.dma_start(xbf[:], x_d[:])
            nc.gpsimd.dma_start(W1bf[:], W1_d[:])
            nc.gpsimd.dma_start(W2bf[:], W2_d[:])

            # constants / small tensors
            hinv = cpool.tile([128, 1], F32)
            nc.sync.dma_start(hinv[:], hinv_d[:])
            mask_sb = cpool.tile([DOUT, FD], F32)
            nc.sync.dma_start(mask_sb[:], mask_d[:])

            # transposed weights (bf16)
            W1T = cpool.tile([DIN, DMID], BF16)          # [128, 256]
            nc.sync.dma_start_transpose(W1T[:], W1bf[:])
            W2T0 = cpool.tile([DIN, DOUT], BF16)         # [128, 16] (j 0:128)
            W2T1 = cpool.tile([DIN, DOUT], BF16)
            nc.sync.dma_start_transpose(W2T0[:], W2bf[:, 0:128])
            nc.sync.dma_start_transpose(W2T1[:], W2bf[:, 128:256])
            xT = cpool.tile([DIN, BC], BF16)             # [128, 64]
            nc.sync.dma_start_transpose(xT[:], xbf[:])

            # ---------- query MLP -> vT [16, 64] (= Zw.T / h) ----------
            prologue_psum = tc.tile_pool(name="ppsum", bufs=3, space="PSUM")
            hpsum = prologue_psum.__enter__()
            small_psum = tc.tile_pool(name="spsum", bufs=1, space="PSUM")
            spsum = small_psum.__enter__()
            pq = spsum.tile([128, 512], F32, tag="qp")
            for j in range(2):
                nc.tensor.matmul(pq[:, 64 * j:64 * j + 64],
                                 W1T[:, 128 * j:128 * j + 128], xT[:])
            HqT = cpool.tile([128, 128], BF16)
            nc.scalar.activation(HqT[:], pq[:, 0:128], AF.Relu)
            pz = spsum.tile([128, 512], F32, tag="qp")
            nc.tensor.matmul(pz[0:DOUT, 0:BC], W2T0[:], HqT[:, 0:64],
                             start=True, stop=False)
            nc.tensor.matmul(pz[0:DOUT, 0:BC], W2T1[:], HqT[:, 64:128],
                             start=False, stop=True)
            vT = cpool.tile([DOUT, BC], F32)
            nc.scalar.activation(vT[:], pz[0:DOUT, 0:BC], AF.Copy,
                                 scale=hinv[0:DOUT, 0:1])

            # replicate v (in (d,b) flat order) to 16 partitions via DRAM bounce
            vf_dram = dram.tile([DOUT, BC], F32)
            nc.sync.dma_start(vf_dram[:], vT[:])
            v_rep = cpool.tile([DOUT, FD], F32)
            vf_flat = vf_dram[:].rearrange("d b -> (d b)").unsqueeze(0)
            nc.sync.dma_start(v_rep[:], vf_flat.to_broadcast((DOUT, FD)))

            # Vstack [32, 1024]: rows 0:16 = mask * v_rep ; rows 16:32 = mask
            Vstack = cpool.tile([2 * DOUT, FD], F32)
            nc.vector.tensor_mul(Vstack[0:DOUT, :], mask_sb[:], v_rep[:])
            nc.sync.dma_start(Vstack[DOUT:2 * DOUT, :], mask_sb[:])

            # ---------- reference MLP over calc_X ----------
            # X cast -> Xbf (HBM->HBM), chunked; transpose-load chunks
            XT = big.tile([DIN, N], BF16)                # [128, 8192] bf16
            CH = 2048
            for c in range(N // CH):
                sl = slice(c * CH, (c + 1) * CH)
                nc.gpsimd.dma_start(Xbf[sl, :], X_d[sl, :])
                nc.sync.dma_start_transpose(XT[:, sl], Xbf[sl, :])

            # mm1: H.T chunks [128, 1024] per j-block, relu-drain to bf16
            HT = big.tile([DIN, 2, N], BF16)             # [128, 2, 8192]
            HCH = 1024
            for c in range(N // HCH):
                for j in range(2):
                    ph = hpsum.tile([128, HCH], F32, tag="ph")   # 2 banks
                    for q in range(HCH // 512):
                        nc.tensor.matmul(
                            ph[:, 512 * q:512 * q + 512],
                            W1T[:, 128 * j:128 * j + 128],
                            XT[:, c * HCH + 512 * q: c * HCH + 512 * q + 512])
                    dst = HT[:, j, c * HCH:(c + 1) * HCH]
                    if (c + j) % 2 == 0:
                        nc.scalar.activation(dst, ph[:], AF.Relu)
                    else:
                        nc.vector.tensor_scalar_max(dst, ph[:], 0.0)

            # mm2: Xw.T packed [128, 2048]; row 32*jj+d, col k*512+c
            #   = Xw.T[d, n] / h,  n = k*2048 + jj*512 + c
            uT = big.tile([128, 4 * 512], F32)
            for k in range(4):
                px = spsum.tile([128, 512], F32, tag="qp")
                for jj in range(4):
                    win = slice(k * 2048 + jj * 512, k * 2048 + jj * 512 + 512)
                    nc.tensor.matmul(px[32 * jj:32 * jj + DOUT, :],
                                     W2T0[:], HT[:, 0, win],
                                     start=True, stop=False,
                                     tile_position=(0, 32 * jj))
                    nc.tensor.matmul(px[32 * jj:32 * jj + DOUT, :],
                                     W2T1[:], HT[:, 1, win],
                                     start=False, stop=True,
                                     tile_position=(0, 32 * jj))
                nc.scalar.activation(uT[:, 512 * k:512 * k + 512], px[:],
                                     AF.Copy, scale=hinv[:, 0:1])

            usq = big.tile([128, 4 * 512], F32)
            nc.vector.scalar_tensor_tensor(usq[:], uT[:], -0.5, uT[:],
                                           op0=ALU.mult, op1=ALU.mult)

            # ustack [32, 8192]: rows 0:16 u.T, rows 16:32 -u^2/2
            ustack = big.tile([2 * DOUT, N], F32)
            for k in range(4):
                for jj in range(4):
                    win = slice(k * 2048 + jj * 512, k * 2048 + jj * 512 + 512)
                    src = slice(512 * k, 512 * k + 512)
                    nc.sync.dma_start(ustack[0:DOUT, win],
                                      uT[32 * jj:32 * jj + DOUT, src])
                    nc.sync.dma_start(ustack[DOUT:2 * DOUT, win],
                                      usq[32 * jj:32 * jj + DOUT, src])

            # ya [128, 64*17]: col 17t = 1.0, cols 17t+1+d = y[128t+p, d]
            ya = big.tile([128, NT * 17], F32)
            nc.vector.memset(ya[:].rearrange("p (t s) -> p t s", s=17)[:, :, 0:1], 1.0)
            nc.sync.dma_start(
                ya[:].rearrange("p (t s) -> p t s", s=17)[:, :, 1:17],
                Y_d[:].rearrange("(t p) d -> p t d", p=128))

            small_psum.__exit__(None, None, None)
            prologue_psum.__exit__(None, None, None)

            # ---------- main loop ----------
            with (
                tc.tile_pool(name="argp", bufs=3, space="PSUM") as argp,
                tc.tile_pool(name="redp", bufs=1, space="PSUM") as redp,
                tc.tile_pool(name="wp", bufs=3) as wp,
            ):
                red0 = redp.tile([17, 512], F32, tag="red0")
                red1 = redp.tile([17, 512], F32, tag="red1")
                yar = ya[:].rearrange("p (t s) -> p t s", s=17)
                for t in range(NT):
                    parg = argp.tile([128, FD], F32)
                    lhs = ustack[:, 128 * t:128 * t + 128]
                    nc.tensor.matmul(parg[:, 0:512], lhs, Vstack[:, 0:512])
                    nc.tensor.matmul(parg[:, 512:1024], lhs, Vstack[:, 512:1024])
                    w = wp.tile([128, FD], F32)
                    nc.scalar.activation(w[:], parg[:], AF.Exp)
                    nc.tensor.matmul(red0[:], yar[:, t, :], w[:, 0:512],
                                     start=(t == 0), stop=(t == NT - 1),
                                     skip_group_check=True)
                    nc.tensor.matmul(red1[:], yar[:, t, :], w[:, 512:1024],
                                     start=(t == 0), stop=(t == NT - 1),
                                     skip_group_check=True)

                # ---------- epilogue ----------
                # copy [17,512] psum halves to SBUF, bounce via DRAM, then
                # gather the numerator diagonal with a linear DRAM AP.
                R0 = cpool.tile([17, 512], F32)
                R1 = cpool.tile([17, 512], F32)
                nc.scalar.activation(R0[:], red0[:], AF.Copy)
                nc.scalar.activation(R1[:], red1[:], AF.Copy)
                r_dram = dram.tile([2, 17, 512], F32)
                nc.sync.dma_start(r_dram[0, :, :], R0[:])
                nc.sync.dma_start(r_dram[1, :, :], R1[:])
                # num[0, 64*d+b] = r_dram[d//8, 1+d, (d%8)*64+b]
                #   flat idx = 512 + (d//8)*12800 + (d%8)*576 + b
                num = cpool.tile([1, FD], F32)
                rten = r_dram[:].tensor
                nc.sync.dma_start(
                    num[:].rearrange("p (dh dl b) -> p dh dl b", dh=2, dl=8),
                    bass.AP(rten, 512, [[1, 1], [12800, 2], [576, 8], [1, 64]]))
                # den[0, 64*d+b] = r_dram[d//8, 0, (d%8)*64+b]
                den = cpool.tile([1, FD], F32)
                nc.sync.dma_start(
                    den[:].rearrange("p (dh dl b) -> p dh dl b", dh=2, dl=8),
                    bass.AP(rten, 0, [[1, 1], [8704, 2], [64, 8], [1, 64]]))
                rec = cpool.tile([1, FD], F32)
                nc.vector.reciprocal(rec[:], den[:])
                res = cpool.tile([1, FD], F32)
                nc.vector.tensor_mul(res[:], num[:], rec[:])
                # res[0, d*64+b] -> y_out[b, d]
                nc.sync.dma_start(out_d[:].rearrange("b d -> d b"), res[:])

    nc.compile()
    return nc


_NC = None


def kernel(**inputs):
    global _NC
    x = np.asarray(inputs["x"], dtype=np.float32)
    calc_X = np.asarray(inputs["calc_X"], dtype=np.float32)
    calc_Y = np.asarray(inputs["calc_Y"], dtype=np.float32)
    W1 = np.asarray(inputs["W1"], dtype=np.float32)
    W2 = np.asarray(inputs["W2"], dtype=np.float32)
    h = np.asarray(inputs["h"], dtype=np.float32)

    hinv = np.full((128, 1), 1.0 / float(h[0]), dtype=np.float32)
    mask = np.zeros((DOUT, FD), dtype=np.float32)
    for d in range(DOUT):
        mask[d, 64 * d:64 * d + 64] = 1.0

    in_maps = []
    for c in range(NCORES):
        in_maps.append({
            "x_slab": np.ascontiguousarray(x[BC * c:BC * (c + 1)]),
            "calc_X": calc_X, "calc_Y": calc_Y,
            "W1": W1, "W2": W2,
            "hinv128": hinv, "mask_db": mask,
        })

    if _NC is None:
        _NC = build_kernel()
    res = run_bass_kernel_spmd(_NC, in_maps, core_ids=list(range(NCORES)))
    out = np.concatenate([res.results[c]["y_out"] for c in range(NCORES)], axis=0)
    return out.astype(np.float32)


if __name__ == "__main__":
    rng = np.random.default_rng(0)
    ins = {
        "x": rng.standard_normal((B, DIN), dtype=np.float32),
        "calc_X": rng.standard_normal((N, DIN), dtype=np.float32),
        "calc_Y": rng.standard_normal((N, DOUT), dtype=np.float32),
        "W1": (rng.standard_normal((DMID, DIN), dtype=np.float32) * DIN ** -0.5),
        "W2": (rng.standard_normal((DOUT, DMID), dtype=np.float32) * DMID ** -0.5),
        "h": np.array([1.5], dtype=np.float32),
    }
    out = kernel(**ins)
    # numpy reference
    def mlp(v):
        return np.maximum(v @ ins["W1"].T, 0.0) @ ins["W2"].T
    Zw = mlp(ins["x"]); Xw = mlp(ins["calc_X"])
    z = (Xw[None] - Zw[:, None]) / ins["h"][0]
    w = np.exp(-0.5 * z * z)
    ref = (w * ins["calc_Y"][None]).sum(1) / w.sum(1)
    rel = np.abs(out - ref).max() / np.abs(ref).max()
    print("rel err:", rel)


# revision 12
# speedup vs baseline: 2.1598x; 2.1598x over previous
"""Nadaraya-Watson kernel regression (retrieval_knn) on 8 NeuronCores.

out[b,d] = sum_n y[n,d] * G((Xw[n,d]-Zw[b,d])/h) / sum_n G(...),
G(z) = exp(-z^2/2); Zw = mlp(x), Xw = mlp(calc_X).

Sharding: data-parallel over the query batch B (64 queries/core);
calc_X / calc_Y / weights replicated.

Per-core plan (all fp32):
  - MLP over calc_X on the PE (weights stationary, X.T streamed) ->
    u.T = Xw.T/h  [16, 8192]   (1/h folded into W2.T on the host)
  - query MLP -> v = Zw.T/h [16, 64]
  - main pass in a [(rep,d)=128 partitions, n=8192 free] layout:
    partition p = r*16+d holds query b=8g+r (g = loop tile) and dim d.
    u rows replicated 8x across partitions once (single DMA); for each
    of 8 query-groups g: z = u - v (tensor_scalar with per-partition v),
    w = exp(-z^2/2) on ACT with accum_out giving the denominator row-sum,
    numerator via scalar_tensor_tensor(w * y) with accum_out.
  - epilogue: reciprocal + multiply, single linear DMA to y_out.
"""
import sys
sys.path.insert(0, '/opt/trn_rl_repo')
import numpy as np
from concourse import bass, tile, bacc, mybir
from concourse.bass_utils import run_bass_kernel_spmd

F32 = mybir.dt.float32
AF = mybir.ActivationFunctionType
ALU = mybir.AluOpType

B, N, DIN, DMID, DOUT = 512, 8192, 128, 256, 16
NCORES = 8
BC = B // NCORES            # 64 queries per core
NG = BC // 8                # 8 query-groups; partition p = r*16+d, b = 8g+r
MMF = 512                   # fp32 matmul moving-operand free-size limit
HCH = 2048                  # psum chunk (4 banks)


def build_kernel(reps=1):
    nc = bacc.Bacc(None, target_bir_lowering=False)

    xT_d = nc.dram_tensor("xT", [DIN, BC], F32, kind="ExternalInput")
    XT_d = nc.dram_tensor("XT", [DIN, N], F32, kind="ExternalInput")
    Y_d = nc.dram_tensor("calc_Y", [N, DOUT], F32, kind="ExternalInput")
    W1T_d = nc.dram_tensor("W1T", [DIN, DMID], F32, kind="ExternalInput")
    W2Ta_d = nc.dram_tensor("W2Ta", [DIN, DOUT], F32, kind="ExternalInput")
    W2Tb_d = nc.dram_tensor("W2Tb", [DIN, DOUT], F32, kind="ExternalInput")
    out_d = nc.dram_tensor("y_out", [BC, DOUT], F32, kind="ExternalOutput")

    with tile.TileContext(nc) as tc:
      for _rep in range(reps):
        with (
            tc.tile_pool(name="dram", bufs=1, space="DRAM") as dram,
            tc.tile_pool(name="const", bufs=1) as cpool,
        ):
            mlp_cm = tc.tile_pool(name="mlppool", bufs=1)
            mlp_pool = mlp_cm.__enter__()
            psum_cm = tc.tile_pool(name="ppsum", bufs=2, space="PSUM")
            psum = psum_cm.__enter__()

            # ---------- loads ----------
            XT = mlp_pool.tile([DIN, N], F32)
            nc.sync.dma_start(XT[:], XT_d[:])
            W1T = cpool.tile([DIN, DMID], F32)
            nc.sync.dma_start(W1T[:], W1T_d[:])
            W2Ta = cpool.tile([DIN, DOUT], F32)
            nc.sync.dma_start(W2Ta[:], W2Ta_d[:])
            W2Tb = cpool.tile([DIN, DOUT], F32)
            nc.sync.dma_start(W2Tb[:], W2Tb_d[:])
            xT = cpool.tile([DIN, BC], F32)
            nc.sync.dma_start(xT[:], xT_d[:])

            # ---------- query MLP: vT [16, 64] = Zw.T / h ----------
            pq = psum.tile([128, HCH], F32, tag="ph")
            for j in range(2):
                nc.tensor.matmul(pq[:, 64 * j:64 * j + 64],
                                 W1T[:, 128 * j:128 * j + 128], xT[:])
            HqT = cpool.tile([128, 128], F32)
            nc.scalar.activation(HqT[:], pq[:, 0:128], AF.Relu)
            pz = psum.tile([128, HCH], F32, tag="ph")
            nc.tensor.matmul(pz[0:DOUT, 0:BC], W2Ta[:], HqT[:, 0:64],
                             start=True, stop=False)
            nc.tensor.matmul(pz[0:DOUT, 0:BC], W2Tb[:], HqT[:, 64:128],
                             start=False, stop=True)
            vT = cpool.tile([DOUT, BC], F32)
            nc.scalar.activation(vT[:], pz[0:DOUT, 0:BC], AF.Copy)
            vf_dram = dram.tile([DOUT, BC], F32)
            nc.sync.dma_start(vf_dram[:], vT[:])
            # v_col [128, NG]: v_col[16r+d, g] = v[d, 8g+r]
            v_col = cpool.tile([128, NG], F32)
            for r in range(8):
                nc.sync.dma_start(
                    v_col[16 * r:16 * r + 16, :],
                    bass.AP(vf_dram[:].tensor, r, [[64, DOUT], [8, NG]]))

            # ---------- reference MLP: u.T [16, 8192] = Xw.T / h ----------
            HT = mlp_pool.tile([DIN, 2, N], F32)
            for c in range(N // HCH):
                for j in range(2):
                    ph = psum.tile([128, HCH], F32, tag="ph")
                    for q in range(HCH // MMF):
                        nc.tensor.matmul(
                            ph[:, MMF * q:MMF * (q + 1)],
                            W1T[:, 128 * j:128 * j + 128],
                            XT[:, c * HCH + MMF * q: c * HCH + MMF * (q + 1)])
                    dst = HT[:, j, c * HCH:(c + 1) * HCH]
                    if (c + j) % 2 == 0:
                        nc.scalar.activation(dst, ph[:], AF.Relu)
                    else:
                        nc.vector.tensor_scalar_max(dst, ph[:], 0.0)

            u_sb = mlp_pool.tile([DOUT, N], F32)
            for c in range(N // HCH):
                pu = psum.tile([128, HCH], F32, tag="ph")
                for q in range(HCH // MMF):
                    win = slice(c * HCH + MMF * q, c * HCH + MMF * (q + 1))
                    dstw = slice(MMF * q, MMF * (q + 1))
                    nc.tensor.matmul(pu[0:DOUT, dstw], W2Ta[:], HT[:, 0, win],
                                     start=True, stop=False)
                    nc.tensor.matmul(pu[0:DOUT, dstw], W2Tb[:], HT[:, 1, win],
                                     start=False, stop=True)
                nc.scalar.activation(u_sb[:, c * HCH:(c + 1) * HCH],
                                     pu[0:DOUT, :], AF.Copy)
            psum_cm.__exit__(None, None, None)
            u_dram = dram.tile([DOUT, N], F32)
            nc.sync.dma_start(u_dram[:], u_sb[:])
            mlp_cm.__exit__(None, None, None)

            # ---------- main pass ----------
            den = cpool.tile([128, NG], F32)
            num = cpool.tile([128, NG], F32)
            with tc.tile_pool(name="mp", bufs=1) as mp:
                # U[16r+d, n] = u[d, n]  (one replicated load for all groups)
                U = mp.tile([128, N], F32, tag="U")
                nc.sync.dma_start(
                    U[:], bass.AP(u_dram[:].tensor, 0,
                                  [[0, 8], [N, DOUT], [1, N]]))
                # Yrep[16r+d, n] = y[n, d]
                Yrep = mp.tile([128, N], F32, tag="Yr")
                for r in range(8):
                    nc.sync.dma_start(
                        Yrep[16 * r:16 * r + 16, :],
                        bass.AP(Y_d[:].tensor, 0, [[1, DOUT], [DOUT, N]]))
                for g in range(NG):
                    t1 = mp.tile([128, N], F32, tag="t1")
                    nc.vector.tensor_scalar(t1[:], U[:], v_col[:, g:g + 1], None,
                                            op0=ALU.subtract)
                    sq = mp.tile([128, N], F32, tag="sq")
                    nc.vector.tensor_mul(sq[:], t1[:], t1[:])
                    w = mp.tile([128, N], F32, tag="t1")
                    nc.scalar.activation(w[:], sq[:], AF.Exp, scale=-0.5,
                                         accum_out=den[:, g:g + 1])
                    wy = mp.tile([128, N], F32, tag="sq")
                    nc.vector.scalar_tensor_tensor(wy[:], w[:], 1.0, Yrep[:],
                                                   op0=ALU.bypass, op1=ALU.mult,
                                                   accum_out=num[:, g:g + 1])

            # ---------- epilogue ----------
            rec = cpool.tile([128, NG], F32)
            nc.vector.reciprocal(rec[:], den[:])
            res = cpool.tile([128, NG], F32)
            nc.vector.tensor_mul(res[:], num[:], rec[:])
            # res[16r+d, g] -> y_out[8g+r, d]: flat idx = 128g + p
            nc.sync.dma_start(
                bass.AP(out_d[:].tensor, 0, [[1, 128], [128, NG]]), res[:])

    nc.compile()
    return nc


_NC = None


def prep_in_maps(inputs):
    x = np.asarray(inputs["x"], dtype=np.float32)
    calc_X = np.asarray(inputs["calc_X"], dtype=np.float32)
    calc_Y = np.ascontiguousarray(np.asarray(inputs["calc_Y"], dtype=np.float32))
    W1 = np.asarray(inputs["W1"], dtype=np.float32)
    W2 = np.asarray(inputs["W2"], dtype=np.float32)
    h = float(np.asarray(inputs["h"], dtype=np.float32).reshape(-1)[0])

    XT = np.ascontiguousarray(calc_X.T)                 # [128, 8192]
    W1T = np.ascontiguousarray(W1.T)                    # [128, 256]
    W2Th = np.ascontiguousarray(W2.T) / h               # [256, 16], 1/h folded
    W2Ta = np.ascontiguousarray(W2Th[0:128])
    W2Tb = np.ascontiguousarray(W2Th[128:256])

    in_maps = []
    for c in range(NCORES):
        xTc = np.ascontiguousarray(x[BC * c:BC * (c + 1)].T)   # [128, 64]
        in_maps.append({
            "xT": xTc, "XT": XT, "calc_Y": calc_Y,
            "W1T": W1T, "W2Ta": W2Ta, "W2Tb": W2Tb,
        })
    return in_maps


def kernel(**inputs):
    global _NC
    in_maps = prep_in_maps(inputs)
    if _NC is None:
        _NC = build_kernel()
    res = run_bass_kernel_spmd(_NC, in_maps, core_ids=list(range(NCORES)))
    out = np.concatenate([res.results[c]["y_out"] for c in range(NCORES)], axis=0)
    return out.astype(np.float32)


if __name__ == "__main__":
    rng = np.random.default_rng(0)
    ins = {
        "x": rng.standard_normal((B, DIN), dtype=np.float32),
        "calc_X": rng.standard_normal((N, DIN), dtype=np.float32),
        "calc_Y": rng.standard_normal((N, DOUT), dtype=np.float32),
        "W1": (rng.standard_normal((DMID, DIN), dtype=np.float32) * DIN ** -0.5),
        "W2": (rng.standard_normal((DOUT, DMID), dtype=np.float32) * DMID ** -0.5),
        "h": np.array([1.5], dtype=np.float32),
    }
    out = kernel(**ins)
    def mlp(v):
        return np.maximum(v @ ins["W1"].T, 0.0) @ ins["W2"].T
    Zw = mlp(ins["x"]); Xw = mlp(ins["calc_X"])
    z = (Xw[None] - Zw[:, None]) / ins["h"][0]
    w = np.exp(-0.5 * z * z)
    ref = (w * ins["calc_Y"][None]).sum(1) / w.sum(1)
    rel = np.abs(out - ref).max() / np.abs(ref).max()
    print("rel err:", rel)


# revision 13
# speedup vs baseline: 3.6678x; 1.6982x over previous
"""Nadaraya-Watson kernel regression (retrieval_knn) on 8 NeuronCores.

out[b,d] = sum_n y[n,d] * G((Xw[n,d]-Zw[b,d])/h) / sum_n G(...),
G(z) = exp(-z^2/2); Zw = mlp(x), Xw = mlp(calc_X).

Sharding: data-parallel over the query batch B (64 queries/core);
calc_X / calc_Y / weights replicated.

Per-core plan (all fp32):
  - MLP over calc_X on the PE (weights stationary, X.T streamed) ->
    u.T = Xw.T/h  [16, 8192]   (1/h folded into W2.T on the host)
  - query MLP -> v = Zw.T/h [16, 64]
  - main pass in a [(rep,d)=128 partitions, n=8192 free] layout:
    partition p = r*16+d holds query b=8g+r (g = loop tile) and dim d.
    u rows replicated 8x across partitions once (single DMA); for each
    of 8 query-groups g: z = u - v (tensor_scalar with per-partition v),
    w = exp(-z^2/2) on ACT with accum_out giving the denominator row-sum,
    numerator via scalar_tensor_tensor(w * y) with accum_out.
  - epilogue: reciprocal + multiply, single linear DMA to y_out.
"""
import sys
sys.path.insert(0, '/opt/trn_rl_repo')
import numpy as np
from concourse import bass, tile, bacc, mybir
from concourse.bass_utils import run_bass_kernel_spmd

F32 = mybir.dt.float32
AF = mybir.ActivationFunctionType
ALU = mybir.AluOpType

B, N, DIN, DMID, DOUT = 512, 8192, 128, 256, 16
NCORES = 8
BC = B // NCORES            # 64 queries per core
NG = BC // 8                # 8 query-groups; partition p = r*16+d, b = 8g+r
MMF = 512                   # fp32 matmul moving-operand free-size limit
HCH = 2048                  # psum chunk (4 banks)


def build_kernel(reps=1):
    nc = bacc.Bacc(None, target_bir_lowering=False)

    xT_d = nc.dram_tensor("xT", [DIN, BC], F32, kind="ExternalInput")
    XT_d = nc.dram_tensor("XT", [DIN, N], F32, kind="ExternalInput")
    Y_d = nc.dram_tensor("calc_YT", [DOUT, N], F32, kind="ExternalInput")
    W1T_d = nc.dram_tensor("W1T", [DIN, DMID], F32, kind="ExternalInput")
    W2Ta_d = nc.dram_tensor("W2Ta", [DIN, DOUT], F32, kind="ExternalInput")
    W2Tb_d = nc.dram_tensor("W2Tb", [DIN, DOUT], F32, kind="ExternalInput")
    out_d = nc.dram_tensor("y_out", [BC, DOUT], F32, kind="ExternalOutput")

    with tile.TileContext(nc) as tc:
      for _rep in range(reps):
        with (
            tc.tile_pool(name="dram", bufs=1, space="DRAM") as dram,
            tc.tile_pool(name="const", bufs=1) as cpool,
        ):
            mlp_cm = tc.tile_pool(name="mlppool", bufs=1)
            mlp_pool = mlp_cm.__enter__()
            psum_cm = tc.tile_pool(name="ppsum", bufs=2, space="PSUM")
            psum = psum_cm.__enter__()

            # ---------- loads ----------
            XT = mlp_pool.tile([DIN, N], F32)
            nc.sync.dma_start(XT[:], XT_d[:])
            W1T = cpool.tile([DIN, DMID], F32)
            nc.sync.dma_start(W1T[:], W1T_d[:])
            W2Ta = cpool.tile([DIN, DOUT], F32)
            nc.sync.dma_start(W2Ta[:], W2Ta_d[:])
            W2Tb = cpool.tile([DIN, DOUT], F32)
            nc.sync.dma_start(W2Tb[:], W2Tb_d[:])
            xT = cpool.tile([DIN, BC], F32)
            nc.sync.dma_start(xT[:], xT_d[:])

            # ---------- query MLP: vT [16, 64] = Zw.T / h ----------
            pq = psum.tile([128, HCH], F32, tag="ph")
            for j in range(2):
                nc.tensor.matmul(pq[:, 64 * j:64 * j + 64],
                                 W1T[:, 128 * j:128 * j + 128], xT[:])
            HqT = cpool.tile([128, 128], F32)
            nc.scalar.activation(HqT[:], pq[:, 0:128], AF.Relu)
            pz = psum.tile([128, HCH], F32, tag="ph")
            nc.tensor.matmul(pz[0:DOUT, 0:BC], W2Ta[:], HqT[:, 0:64],
                             start=True, stop=False)
            nc.tensor.matmul(pz[0:DOUT, 0:BC], W2Tb[:], HqT[:, 64:128],
                             start=False, stop=True)
            vT = cpool.tile([DOUT, BC], F32)
            nc.scalar.activation(vT[:], pz[0:DOUT, 0:BC], AF.Copy)
            vf_dram = dram.tile([DOUT, BC], F32)
            nc.sync.dma_start(vf_dram[:], vT[:])
            # v_col [128, NG]: v_col[16r+d, g] = v[d, 8g+r]
            v_col = cpool.tile([128, NG], F32)
            for r in range(8):
                nc.sync.dma_start(
                    v_col[16 * r:16 * r + 16, :],
                    bass.AP(vf_dram[:].tensor, r, [[64, DOUT], [8, NG]]))

            # ---------- reference MLP: u.T [16, 8192] = Xw.T / h ----------
            HT = mlp_pool.tile([DIN, 2, N], F32)
            for c in range(N // HCH):
                for j in range(2):
                    ph = psum.tile([128, HCH], F32, tag="ph")
                    for q in range(HCH // MMF):
                        nc.tensor.matmul(
                            ph[:, MMF * q:MMF * (q + 1)],
                            W1T[:, 128 * j:128 * j + 128],
                            XT[:, c * HCH + MMF * q: c * HCH + MMF * (q + 1)])
                    dst = HT[:, j, c * HCH:(c + 1) * HCH]
                    if (c + j) % 2 == 0:
                        nc.scalar.activation(dst, ph[:], AF.Relu)
                    else:
                        nc.vector.tensor_scalar_max(dst, ph[:], 0.0)

            u_sb = mlp_pool.tile([DOUT, N], F32)
            for c in range(N // HCH):
                pu = psum.tile([128, HCH], F32, tag="ph")
                for q in range(HCH // MMF):
                    win = slice(c * HCH + MMF * q, c * HCH + MMF * (q + 1))
                    dstw = slice(MMF * q, MMF * (q + 1))
                    nc.tensor.matmul(pu[0:DOUT, dstw], W2Ta[:], HT[:, 0, win],
                                     start=True, stop=False)
                    nc.tensor.matmul(pu[0:DOUT, dstw], W2Tb[:], HT[:, 1, win],
                                     start=False, stop=True)
                nc.scalar.activation(u_sb[:, c * HCH:(c + 1) * HCH],
                                     pu[0:DOUT, :], AF.Copy)
            psum_cm.__exit__(None, None, None)
            u_dram = dram.tile([DOUT, N], F32)
            nc.sync.dma_start(u_dram[:], u_sb[:])
            usqh_sb = mlp_pool.tile([DOUT, N], F32)
            nc.vector.scalar_tensor_tensor(usqh_sb[:], u_sb[:], -0.5, u_sb[:],
                                           op0=ALU.mult, op1=ALU.mult)
            usqh_dram = dram.tile([DOUT, N], F32)
            nc.sync.dma_start(usqh_dram[:], usqh_sb[:])
            mlp_cm.__exit__(None, None, None)

            # ---------- main pass ----------
            den = cpool.tile([128, NG], F32)
            num = cpool.tile([128, NG], F32)
            with tc.tile_pool(name="mp", bufs=1) as mp:
                # U[16r+d, n] = u[d, n]  (one replicated load for all groups)
                U = mp.tile([128, N], F32, tag="U")
                nc.sync.dma_start(
                    U[:], bass.AP(u_dram[:].tensor, 0,
                                  [[0, 8], [N, DOUT], [1, N]]))
                # Yrep[16r+d, n] = y[n, d]
                Yrep = mp.tile([128, N], F32, tag="Yr")
                nc.sync.dma_start(
                    Yrep[:], bass.AP(Y_d[:].tensor, 0,
                                     [[0, 8], [N, DOUT], [1, N]]))
                # USQH[16r+d, n] = -u[d, n]^2/2
                USQH = mp.tile([128, N], F32, tag="Uq")
                nc.sync.dma_start(
                    USQH[:], bass.AP(usqh_dram[:].tensor, 0,
                                     [[0, 8], [N, DOUT], [1, N]]))
                for g in range(NG):
                    # arg = u*v - u^2/2  (e^{-v^2/2} factor cancels in the ratio)
                    sq = mp.tile([128, N], F32, tag="sq")
                    nc.vector.scalar_tensor_tensor(sq[:], U[:], v_col[:, g:g + 1],
                                                   USQH[:], op0=ALU.mult,
                                                   op1=ALU.add)
                    w = mp.tile([128, N], F32, tag="w")
                    nc.scalar.activation(w[:], sq[:], AF.Exp,
                                         accum_out=den[:, g:g + 1])
                    wy = mp.tile([128, N], F32, tag="sq")
                    nc.vector.scalar_tensor_tensor(wy[:], w[:], 1.0, Yrep[:],
                                                   op0=ALU.bypass, op1=ALU.mult,
                                                   accum_out=num[:, g:g + 1])

            # ---------- epilogue ----------
            rec = cpool.tile([128, NG], F32)
            nc.vector.reciprocal(rec[:], den[:])
            res = cpool.tile([128, NG], F32)
            nc.vector.tensor_mul(res[:], num[:], rec[:])
            # res[16r+d, g] -> y_out[8g+r, d]: flat idx = 128g + p
            nc.sync.dma_start(
                bass.AP(out_d[:].tensor, 0, [[1, 128], [128, NG]]), res[:])

    nc.compile()
    return nc


_NC = None


def prep_in_maps(inputs):
    x = np.asarray(inputs["x"], dtype=np.float32)
    calc_X = np.asarray(inputs["calc_X"], dtype=np.float32)
    calc_Y = np.ascontiguousarray(np.asarray(inputs["calc_Y"], dtype=np.float32))
    W1 = np.asarray(inputs["W1"], dtype=np.float32)
    W2 = np.asarray(inputs["W2"], dtype=np.float32)
    h = float(np.asarray(inputs["h"], dtype=np.float32).reshape(-1)[0])

    XT = np.ascontiguousarray(calc_X.T)                 # [128, 8192]
    YT = np.ascontiguousarray(calc_Y.T)                 # [16, 8192]
    W1T = np.ascontiguousarray(W1.T)                    # [128, 256]
    W2Th = np.ascontiguousarray(W2.T) / h               # [256, 16], 1/h folded
    W2Ta = np.ascontiguousarray(W2Th[0:128])
    W2Tb = np.ascontiguousarray(W2Th[128:256])

    in_maps = []
    for c in range(NCORES):
        xTc = np.ascontiguousarray(x[BC * c:BC * (c + 1)].T)   # [128, 64]
        in_maps.append({
            "xT": xTc, "XT": XT, "calc_YT": YT,
            "W1T": W1T, "W2Ta": W2Ta, "W2Tb": W2Tb,
        })
    return in_maps


def kernel(**inputs):
    global _NC
    in_maps = prep_in_maps(inputs)
    if _NC is None:
        _NC = build_kernel()
    res = run_bass_kernel_spmd(_NC, in_maps, core_ids=list(range(NCORES)))
    out = np.concatenate([res.results[c]["y_out"] for c in range(NCORES)], axis=0)
    return out.astype(np.float32)


if __name__ == "__main__":
    rng = np.random.default_rng(0)
    ins = {
        "x": rng.standard_normal((B, DIN), dtype=np.float32),
        "calc_X": rng.standard_normal((N, DIN), dtype=np.float32),
        "calc_Y": rng.standard_normal((N, DOUT), dtype=np.float32),
        "W1": (rng.standard_normal((DMID, DIN), dtype=np.float32) * DIN ** -0.5),
        "W2": (rng.standard_normal((DOUT, DMID), dtype=np.float32) * DMID ** -0.5),
        "h": np.array([1.5], dtype=np.float32),
    }
    out = kernel(**ins)
    def mlp(v):
        return np.maximum(v @ ins["W1"].T, 0.0) @ ins["W2"].T
    Zw = mlp(ins["x"]); Xw = mlp(ins["calc_X"])
    z = (Xw[None] - Zw[:, None]) / ins["h"][0]
    w = np.exp(-0.5 * z * z)
    ref = (w * ins["calc_Y"][None]).sum(1) / w.sum(1)
    rel = np.abs(out - ref).max() / np.abs(ref).max()
    print("rel err:", rel)
